# revision 45
# baseline (speedup 1.0000x reference)
"""Trainium2 Bass kernel for nn_CausalSelfAttention_39213051412899.

Sliding-window causal GQA attention with value-embedding gate.
Sharding: 8 cores = batch(2) x kv-group(4).  Each core computes its batch's
4 query heads / 1 kv head and a row-parallel partial of the output
projection; the host sums the 4 partials per batch.

v2: restructured for PE density (HAM stays warm) + bf16 operands.
 - QKV projection emitted per sequence-half; rope/RMS/gate (DVE/ACT) for
   half 0 overlap the projection matmuls of half 1, attention chunk 0
   overlaps phase 2 of half 1.
 - q/k transposed via DMA XBAR transpose (bf16) instead of PE transposes.
 - k's RMS norm is folded into the exp() per-partition scale, so k is
   never explicitly normalized.
 - window masks are 0/1 multiplies on GpSimd after exp (SBUF only),
   keeping DVE/ACT off the PSUM ports.
 - softmax denominator: ones-column in the PV stationary; 1/L applied
   via reciprocal + SBUF broadcast DMA + one fused multiply per chunk.
"""

import os
import sys

import numpy as np

try:
    import concourse.bass as bass  # noqa: F401
except ImportError:  # pragma: no cover
    sys.path.insert(0, "/opt/trn_rl_repo")

import concourse.bass as bass
import concourse.tile as tile
from concourse import bacc
from concourse import mybir
from concourse.bass_utils import run_bass_kernel_spmd

F32 = mybir.dt.float32
BF16 = mybir.dt.bfloat16
AF = mybir.ActivationFunctionType
ALU = mybir.AluOpType

B, S, E = 2, 2048, 1024
H, KV, D = 16, 4, 64
G = H // KV          # 4 q heads per kv head (per core)
GC = 128             # gate channels
EPS = 1.1920929e-07
T = S // 128         # 16 s-tiles
QKVW = 388           # q(256) | k(64) | v(64) | gate(1) | pad(3)
GATE_COL = 384
NCORES = 8
HT = T // 2          # tiles per half

_cache = {}
last_results = None   # test harness reads exec_time_ns off this


def _block_range(ti, tj, lo_delta, hi_delta):
    """Active query-tile range for key-tile tj (None if empty)."""
    lo = max(ti[0], tj - hi_delta)
    hi = min(ti[1], tj - lo_delta)
    if lo > hi:
        return None
    return lo, hi


def _mask_table(wl, wr_eff):
    """Distinct 0/1 mask tiles needed, keyed (kind, base) -> index."""
    lo_delta = -((127 + wl) // 128)
    hi_delta = (127 + wr_eff) // 128
    keys = {}
    for dt_ in range(lo_delta, hi_delta + 1):   # dt_ = tj - tb
        bw = wl + 128 * dt_
        if -127 <= bw < 127:
            keys.setdefault(("w", bw), len(keys))
        bc = wr_eff - 128 * dt_
        if -127 <= bc < 127:
            keys.setdefault(("c", bc), len(keys))
    return keys


def _mask_tiles(wl, wr_eff):
    keys = _mask_table(wl, wr_eff)
    n = max(1, len(keys))
    m = np.ones((128, n * 128), np.float32)
    rj = np.arange(128)[:, None]
    ri = np.arange(128)[None, :]
    for (kind, base), i in keys.items():
        if kind == "w":
            bad = (rj - ri + base) < 0
        else:
            bad = (ri - rj + base) < 0
        m[:, i * 128:(i + 1) * 128] = np.where(bad, 0.0, 1.0)
    return m


def _build(wl, wr):
    wr_eff = min(int(wr), 0)
    wl = int(wl)
    lo_delta = -((127 + wl) // 128)    # tj - ti >= lo_delta
    hi_delta = (127 + wr_eff) // 128   # tj - ti <= hi_delta  (0 when wr>=0)

    nc = bacc.Bacc(None, target_bir_lowering=False)
    d_xT = nc.declare_dram_parameter("xT", [E, S], BF16, isOutput=False)
    d_wqkv = nc.declare_dram_parameter("wqkv", [E, QKVW], BF16, isOutput=False)
    d_cs5 = nc.declare_dram_parameter("cos5", [128, T * 160], BF16,
                                      isOutput=False)
    d_sn5 = nc.declare_dram_parameter("sin5", [128, T * 160], BF16,
                                      isOutput=False)
    d_ve2 = nc.declare_dram_parameter("ve2", [128, T * D], BF16,
                                      isOutput=False)
    d_wproj = nc.declare_dram_parameter("wproj", [G * D, E], BF16,
                                        isOutput=False)
    mask_idx = _mask_table(wl, wr_eff)
    nmask = max(1, len(mask_idx))
    d_masks = nc.declare_dram_parameter("masks", [128, nmask * 128], BF16,
                                        isOutput=False)
    d_ident = nc.declare_dram_parameter("ident", [128, 128], BF16,
                                        isOutput=False)
    d_out = nc.declare_dram_parameter("outp", [S, E], BF16, isOutput=True)

    with tile.TileContext(nc) as tc:
        with tc.tile_pool(name="persist", bufs=1) as persist:
            qT = [persist.tile([128, S], BF16, tag=f"qT{i}", name=f"qT{i}")
                  for i in range(2)]                       # 2 heads/tile
            kT = persist.tile([128, S], BF16)              # k duplicated
            vaug = persist.tile([128, T, D + 1], BF16)     # v | ones col
            yT = [persist.tile([128, S], BF16, tag=f"yT{i}", name=f"yT{i}")
                  for i in range(2)]
            sig = persist.tile([128, T], F32)
            statqk = persist.tile([128, T * 5], F32)  # q heads 0-3 | k
            rss = persist.tile([128, T * 5], F32)
            rsq = persist.tile([128, T * G], F32)   # 1/rms_q
            rsk = persist.tile([128, T], F32)       # D^-.5/rms_k (exp scale)
            masks = persist.tile([128, nmask * 128], BF16)
            ident = persist.tile([128, 128], BF16)
            wp_s = persist.tile([128, 2, E], BF16)
            epsq = persist.tile([128, 1], F32)
            epsk = persist.tile([128, 1], F32)

            nc.vector.memset(epsq[:], EPS)
            nc.vector.memset(epsk[:], EPS * D)
            nc.vector.memset(vaug[:, :, D], 1.0)

            # ---------------- phase 1+2: projections + prep per half -------
            with (
                tc.tile_pool(name="ph1", bufs=1) as ph1,
                tc.tile_pool(name="ph1x", bufs=2) as ph1x,
                tc.tile_pool(name="ph1s", bufs=2) as ph1s,
                tc.tile_pool(name="ph1t", bufs=2) as ph1t,
                tc.tile_pool(name="pq", bufs=3, space="PSUM") as pq,
                tc.tile_pool(name="pw", bufs=1, space="PSUM") as pw,
                tc.tile_pool(name="ptr", bufs=3, space="PSUM") as ptr,
            ):
                wq_s = ph1.tile([128, 8, QKVW], BF16)
                cs5 = ph1.tile([128, T * 160], BF16)
                sn5 = ph1.tile([128, T * 160], BF16)
                ve2 = ph1.tile([128, T * D], BF16)
                # DMA priority: weights + half-0 activations on the sync
                # queue (needed first), everything else on the scalar queue
                xtss = [ph1x.tile([128, 8, HT * 128], BF16, tag="xts",
                                  name=f"xts{i}") for i in range(2)]
                for c in range(8):
                    nc.sync.dma_start(wq_s[:, c, :],
                                      d_wqkv[c * 128:(c + 1) * 128, :])
                    eng = nc.sync if c % 2 == 0 else nc.scalar
                    eng.dma_start(
                        xtss[0][:, c, :],
                        d_xT[c * 128:(c + 1) * 128, 0:HT * 128])
                nc.sync.dma_start(cs5[:], d_cs5[:, :])
                nc.sync.dma_start(sn5[:], d_sn5[:, :])
                for c in range(8):
                    eng = nc.sync if c % 2 == 0 else nc.scalar
                    eng.dma_start(
                        xtss[1][:, c, :],
                        d_xT[c * 128:(c + 1) * 128, HT * 128:S])
                nc.scalar.dma_start(ve2[:], d_ve2[:, :])
                nc.scalar.dma_start(masks[:], d_masks[:, :])
                nc.scalar.dma_start(ident[:], d_ident[:, :])
                for kc in range(2):
                    nc.scalar.dma_start(wp_s[:, kc, :],
                                        d_wproj[kc * 128:(kc + 1) * 128, :])

                # HAM warm-up: keep the PE busy from t=0 so it reaches
                # full clock before the projection starts (and stays there)
                warm = ph1.tile([128, 512], BF16)
                nc.vector.memset(warm[:], 0.25)
                wps = pw.tile([128, 512], F32)
                for _ in range(14):
                    nc.tensor.matmul(wps[:], warm[:, 0:128], warm[:],
                                     start=True, stop=True)

                rqks = []
                for hf in range(2):
                    t0 = hf * HT
                    xts = xtss[hf]
                    qkv = ph1s.tile([128, HT, QKVW], BF16, tag="qkv")
                    for t in range(HT):
                        ps = pq.tile([128, QKVW], F32)
                        for c in range(8):
                            nc.tensor.matmul(
                                ps[:], (xts[:, c, t * 128:(t + 1) * 128]),
                                (wq_s[:, c, :]),
                                start=(c == 0), stop=(c == 7))
                        nc.scalar.copy(qkv[:, t, :], ps[:])

                    # ----- phase 2 (this half): gate, rope, stats, norm ----
                    # sub-batched over 4-tile groups so downstream work can
                    # start before the whole half's projection finishes
                    rqk = ph1t.tile([128, HT, 6, D], BF16, tag="rqk")
                    rqks.append((qkv, rqk))
                    SB = HT // 4
                    for sb in range(4):
                        s0 = sb * SB
                        ts_ = slice(s0, s0 + SB)
                        g0 = t0 + s0
                        nc.scalar.activation(
                            sig[:, g0:g0 + SB],
                            qkv[:, ts_, GATE_COL:GATE_COL + 1].rearrange(
                                "p t o -> p (t o)"),
                            AF.Sigmoid)

                        # rope over q(4 heads) + k: cols 0:320 = 5x64
                        qk5 = qkv[:, ts_, 0:5 * D].rearrange(
                            "p t (h d) -> p t h d", h=5)
                        cosv = cs5[:, g0 * 160:(g0 + SB) * 160].rearrange(
                            "p (t h f) -> p t h f", h=5, f=32)
                        sinv = sn5[:, g0 * 160:(g0 + SB) * 160].rearrange(
                            "p (t h f) -> p t h f", h=5, f=32)
                        tmp = ph1t.tile([128, SB, 5, 32], BF16, tag="tmp")
                        r5 = rqk[:, ts_, 0:5, :]
                        x1 = qk5[:, :, :, 0:32]
                        x2 = qk5[:, :, :, 32:64]
                        nc.vector.tensor_mul(r5[:, :, :, 0:32], x1, cosv)
                        nc.vector.tensor_mul(tmp[:], x2, sinv)
                        nc.vector.tensor_add(
                            r5[:, :, :, 0:32], r5[:, :, :, 0:32], tmp[:])
                        nc.vector.tensor_mul(r5[:, :, :, 32:64], x2, cosv)
                        nc.vector.tensor_mul(tmp[:], x1, sinv)
                        nc.vector.tensor_sub(
                            r5[:, :, :, 32:64], r5[:, :, :, 32:64], tmp[:])

                        # rms stats from roped q/k (rotation-invariant)
                        sq = ph1t.tile([128, SB, 5, D], BF16, tag="sq")
                        nc.vector.tensor_mul(sq[:], r5, r5)
                        sqk = statqk[:, g0 * 5:(g0 + SB) * 5].rearrange(
                            "p (t h) -> p t h", h=5)
                        nc.vector.tensor_reduce(
                            op=ALU.add, out=sqk, in_=sq[:],
                            axis=mybir.AxisListType.X)

                        # v' = v + sig*ve2 -> vaug, on GpSimd (off the
                        # DVE critical chain; all operands in SBUF)
                        for t in range(s0, s0 + SB):
                            tt = t0 + t
                            nc.gpsimd.tensor_scalar_mul(
                                rqk[:, t, 5, :], ve2[:, tt * D:(tt + 1) * D],
                                sig[:, tt:tt + 1])

                        # q: 1/sqrt(ssq/D+eps); k: D^-.5 folded into scale
                        rssv = rss[:, g0 * 5:(g0 + SB) * 5].rearrange(
                            "p (t h) -> p t h", h=5)
                        nc.scalar.activation(
                            rssv[:, :, 0:4], sqk[:, :, 0:4],
                            AF.Sqrt, bias=epsq[:], scale=1.0 / D)
                        nc.scalar.activation(
                            rssv[:, :, 4:5], sqk[:, :, 4:5],
                            AF.Sqrt, bias=epsk[:], scale=1.0)
                        nc.vector.reciprocal(
                            rsq[:, g0 * G:(g0 + SB) * G].rearrange(
                                "p (t h) -> p t h", h=G),
                            rssv[:, :, 0:4])
                        nc.vector.reciprocal(
                            rsk[:, g0:g0 + SB].rearrange(
                                "p (t h) -> p t h", h=1),
                            rssv[:, :, 4:5])

                        # normalize q in place (k stays unnormalized):
                        # one broadcast multiply per sub-batch
                        rsqv = rsq[:, g0 * G:(g0 + SB) * G].rearrange(
                            "p (t h) -> p t h", h=G)
                        nc.vector.tensor_mul(
                            rqk[:, ts_, 0:4, :], rqk[:, ts_, 0:4, :],
                            rsqv.to_broadcast([128, SB, G, D]))

                        nc.gpsimd.tensor_add(
                            vaug[:, t0 + s0:t0 + s0 + SB, 0:D],
                            rqk[:, ts_, 5, :], qkv[:, ts_, 5 * D:6 * D])

                # transposes on the PE (after both halves' projections so
                # the PE never waits on phase-2 DVE work mid-projection)
                for hf in range(2):
                    t0 = hf * HT
                    qkv, rqk = rqks[hf]
                    for t in range(HT):
                        tt = t0 + t
                        for bk, (c0_, c1_) in enumerate(
                                ((0, 2), (2, 4), (3, 5))):
                            tp = ptr.tile([128, 128], BF16, tag="tp")
                            nc.tensor.transpose(
                                (tp[:]),
                                (rqk[:, t, c0_:c1_, :].rearrange(
                                    "p a b -> p (a b)")),
                                (ident[:]))
                            dst = (qT[bk] if bk < 2 else kT)
                            nc.scalar.copy(
                                dst[:, tt * 128:(tt + 1) * 128], tp[:])
                    # rows 64:127 of kT hold k; copy down for even heads
                    nc.sync.dma_start(
                        kT[0:64, t0 * 128:(t0 + HT) * 128],
                        kT[64:128, t0 * 128:(t0 + HT) * 128])

            # ---------------- phase 3: attention ---------------------------
            def _has_full(cs_tiles):
                for C in range(T // cs_tiles):
                    ti_ = (cs_tiles * C, cs_tiles * C + cs_tiles - 1)
                    if not any(_block_range(ti_, tj, lo_delta, hi_delta) == ti_
                               for tj in range(T)):
                        return False
                return True
            CST = 8 if _has_full(8) else 4       # chunk tiles
            CS = CST * 128
            NCH = T // CST
            pad_mode = not _has_full(CST)

            with (
                tc.tile_pool(name="att", bufs=8) as att,
                tc.tile_pool(name="plb", bufs=4) as plb,
                tc.tile_pool(name="pln", bufs=4) as pln,
                tc.tile_pool(name="ytu", bufs=4) as pytu,
                tc.tile_pool(name="ytn", bufs=2) as pytn,
                tc.tile_pool(name="psc", bufs=2, space="PSUM") as psc,
                tc.tile_pool(name="ppv", bufs=2, space="PSUM") as ppv,
                tc.tile_pool(name="dsc", bufs=1, space="DRAM") as dsc,
            ):
                d_linv = dsc.tile([G * NCH, CS], F32)
                def chunk_steps(h, C):
                    rh = slice((h % 2) * 64, (h % 2) * 64 + 64)
                    qTh = qT[h // 2]
                    c0 = CST * C
                    ti = (c0, c0 + CST - 1)
                    tjs = [tj for tj in
                           range(max(0, c0 + lo_delta),
                                 min(T - 1, c0 + CST - 1 + hi_delta) + 1)
                           if _block_range(ti, tj, lo_delta, hi_delta)]
                    full = [tj for tj in tjs
                            if _block_range(ti, tj, lo_delta, hi_delta) == ti]
                    if pad_mode:
                        order = tjs
                    else:
                        ftj = full[-1]
                        order = [ftj] + [tj for tj in tjs if tj != ftj]

                    yTa = ppv.tile([65, CS], F32, tag="yTa", name="yTa")
                    half_started = [False] * (CST // 4)
                    half_last = {}
                    for i, tj in enumerate(order):
                        lo_, hi_ = _block_range(ti, tj, lo_delta, hi_delta)
                        o_, n_ = ((0, CS) if (pad_mode and i == 0) else
                                  ((lo_ - c0) * 128, (hi_ - lo_ + 1) * 128))
                        for hx in range(CST // 4):
                            if o_ < (hx + 1) * 512 and o_ + n_ > hx * 512:
                                half_last[hx] = i
                    pend = []

                    def emit_pv(rec):
                        i, tj, pt, off, n = rec
                        for hx in range(CST // 4):
                            h0 = hx * 512
                            s0 = max(off, h0)
                            s1 = min(off + n, h0 + 512)
                            if s0 >= s1:
                                continue
                            first = not half_started[hx]
                            half_started[hx] = True
                            nc.tensor.matmul(
                                yTa[:, s0:s1], vaug[:, tj, :],
                                pt[:, s0:s1],
                                start=first, stop=(half_last[hx] == i))

                    for i, tj in enumerate(order):
                        alo, ahi = _block_range(ti, tj, lo_delta, hi_delta)
                        aoff = (alo - c0) * 128
                        an = (ahi - alo + 1) * 128
                        if pad_mode and i == 0:
                            off, n = 0, CS
                        else:
                            off, n = aoff, an
                        sc = psc.tile([128, CS], F32, tag="sc", name="sc")
                        pt = att.tile([128, CS], BF16, tag="pt", name="pt")
                        p0 = aoff
                        while p0 < aoff + an:
                            p1 = min((p0 // 512 + 1) * 512, aoff + an)
                            nc.tensor.matmul(
                                sc[:, p0:p1],
                                kT[rh, tj * 128:(tj + 1) * 128],
                                qTh[rh, C * CS + p0:C * CS + p1],
                                start=True, stop=True)
                            p0 = p1
                        if pad_mode:
                            nc.vector.memset(pt[:], 0.0)
                        nc.scalar.activation(
                            pt[:, aoff:aoff + an], sc[:, aoff:aoff + an],
                            AF.Exp, scale=rsk[:, tj:tj + 1])
                        # 0/1 window masks on GpSimd (post-exp, SBUF only)
                        for tb in range(alo, ahi + 1):
                            bo = (tb - c0) * 128
                            for kind, base in (("w", wl - 128 * (tb - tj)),
                                               ("c", wr_eff + 128 * (tb - tj))):
                                if -127 <= base < 127:
                                    mi = mask_idx[(kind, base)]
                                    nc.gpsimd.tensor_mul(
                                        pt[:, bo:bo + 128],
                                        pt[:, bo:bo + 128],
                                        masks[:, mi * 128:(mi + 1) * 128])
                        pend.append((i, tj, pt, off, n))
                        if len(pend) > 1:
                            emit_pv(pend.pop(0))
                        yield
                    emit_pv(pend.pop(0))

                    # 1/L: reciprocal of ones-row, broadcast, fused scale
                    # release PSUM fast: 1/L on ACT + raw-y copy on DVE;
                    # the broadcast + final scale happen off critical path
                    rr = h * NCH + C
                    ln0 = pln.tile([1, CS], F32, tag="ln0")
                    nc.vector.tensor_copy(ln0[:], yTa[64:65, :])
                    ln = pln.tile([1, CS], F32, tag="ln")
                    nc.vector.reciprocal_approx_fast(ln[:], ln0[:])
                    ytu = pytu.tile([64, CS], BF16, tag="ytu")
                    with nc.allow_low_precision(reason="bf16 y"):
                        nc.vector.tensor_copy(ytu[:], yTa[0:64, :])
                    nc.sync.dma_start(d_linv[rr:rr + 1, :], ln[0:1, :])
                    lb = plb.tile([64, CS], F32)
                    nc.sync.dma_start(
                        lb[:], d_linv[rr:rr + 1, :].to_broadcast([64, CS]))
                    ccols = slice(C * CS, (C + 1) * CS)
                    with nc.allow_low_precision(reason="bf16 yT"):
                        if h % 2 == 0:
                            nc.vector.tensor_mul(
                                yT[h // 2][0:64, ccols], ytu[:], lb[:])
                        else:
                            ytn = pytn.tile([64, CS], BF16)
                            nc.vector.tensor_mul(ytn[:], ytu[:], lb[:])
                            nc.sync.dma_start(yT[h // 2][64:128, ccols],
                                              ytn[:])
                    yield

                # drain pairs of equal-length streams interleaved so PE
                # always has an independent chunk to work on
                pairs = []
                for C in range(NCH):
                    for h in range(0, G, 2):
                        pairs.append((chunk_steps(h, C),
                                      chunk_steps(h + 1, C)))
                for gpair in pairs:
                    active = list(gpair)
                    while active:
                        for g in list(active):
                            try:
                                next(g)
                            except StopIteration:
                                active.remove(g)

            # ---------------- phase 4: output projection -------------------
            with (
                tc.tile_pool(name="ob", bufs=4) as pob,
                tc.tile_pool(name="po", bufs=4, space="PSUM") as ppo,
            ):
                for t in range(T):
                    po = ppo.tile([128, E], F32)
                    for nh in range(2):
                        cols = slice(nh * 512, (nh + 1) * 512)
                        nc.tensor.matmul(
                            po[:, cols], (yT[0][:, t * 128:(t + 1) * 128]),
                            (wp_s[:, 0, cols]),
                            start=True, stop=False)
                        nc.tensor.matmul(
                            po[:, cols], (yT[1][:, t * 128:(t + 1) * 128]),
                            (wp_s[:, 1, cols]),
                            start=False, stop=True)
                    ob = pob.tile([128, E], BF16)
                    with nc.allow_low_precision(reason="bf16 out"):
                        if t % 2 == 0:
                            nc.vector.tensor_copy(ob[:], po[:])
                        else:
                            nc.scalar.copy(ob[:], po[:])
                    eng = nc.sync if t % 2 == 0 else nc.scalar
                    eng.dma_start(d_out[t * 128:(t + 1) * 128, :], ob[:])
    nc.compile()
    return nc


def _prep_inputs(x, ve, cos, sin, Wq, Wk, Wv, Wproj, Wgate):
    import ml_dtypes
    bf16 = ml_dtypes.bfloat16

    cosn = np.asarray(cos, np.float32).reshape(S, 32)
    sinn = np.asarray(sin, np.float32).reshape(S, 32)
    cs5 = np.empty((128, T * 160), np.float32)
    sn5 = np.empty((128, T * 160), np.float32)
    for t in range(T):
        cs5[:, t * 160:(t + 1) * 160] = np.tile(
            cosn[t * 128:(t + 1) * 128], (1, 5))
        sn5[:, t * 160:(t + 1) * 160] = np.tile(
            sinn[t * 128:(t + 1) * 128], (1, 5))

    Wq = np.asarray(Wq, np.float32)
    Wk = np.asarray(Wk, np.float32)
    Wv = np.asarray(Wv, np.float32)
    Wproj = np.asarray(Wproj, np.float32)
    Wgate = np.asarray(Wgate, np.float32)
    maps = []
    wl_ = int(getattr(_prep_inputs, '_wl', 1024))
    wr_ = min(int(getattr(_prep_inputs, '_wr', 0)), 0)
    maskt = _mask_tiles(wl_, wr_).astype(bf16)
    cs5 = cs5.astype(bf16)
    sn5 = sn5.astype(bf16)
    for core in range(NCORES):
        b, g = core // 4, core % 4
        xT = np.ascontiguousarray(
            np.asarray(x[b], np.float32).T).astype(bf16)
        wg = np.zeros((E, 1), np.float32)
        wg[:GC, 0] = Wgate[:, g]
        wqkv = np.ascontiguousarray(np.concatenate([
            Wq[:, g * G * D:(g + 1) * G * D],
            Wk[:, g * D:(g + 1) * D],
            Wv[:, g * D:(g + 1) * D],
            wg, np.zeros((E, 3), np.float32)], axis=1)).astype(bf16)
        veg = 2.0 * np.asarray(ve[b][:, g * D:(g + 1) * D], np.float32)
        ve2 = np.ascontiguousarray(
            veg.reshape(T, 128, D).transpose(1, 0, 2).reshape(
                128, T * D)).astype(bf16)
        wproj = np.ascontiguousarray(
            Wproj[g * G * D:(g + 1) * G * D, :]).astype(bf16)
        maps.append({"xT": xT, "wqkv": wqkv, "cos5": cs5, "sin5": sn5,
                     "ve2": ve2, "wproj": wproj, "masks": maskt,
                     "ident": np.eye(128, dtype=np.float32).astype(bf16)})
    return maps


def kernel(x, ve, cos, sin, Wq, Wk, Wv, Wproj, Wgate,
           window_left, window_right):
    global last_results
    wl, wr = int(window_left), int(window_right)
    key = (wl, wr)
    if key not in _cache:
        _cache[key] = _build(wl, wr)
    nc = _cache[key]
    _prep_inputs._wl, _prep_inputs._wr = wl, wr
    maps = _prep_inputs(x, ve, cos, sin, Wq, Wk, Wv, Wproj, Wgate)
    res = run_bass_kernel_spmd(
        nc, maps, core_ids=list(range(NCORES)),
        trace=bool(int(os.environ.get("KERNEL_TRACE", "0"))))
    last_results = res
    out = np.zeros((B, S, E), np.float32)
    for core in range(NCORES):
        out[core // 4] += np.asarray(res.results[core]["outp"], np.float32)
    return out
